# revision 1
# baseline (speedup 1.0000x reference)
"""Top-1 MoE block (B=4, S=2048, H=2048, E=8) for 8 Trainium2 NeuronCores.

Strategy (expert-parallel, host-mediated dispatch):
  - Host computes the tiny gating network (x @ Wg -> softmax -> argmax),
    0.4% of total FLOPs, and the token permutation per expert.
  - Token block for expert e (prob-scaled, transposed to [H, C]) plus W[e]
    goes to core e.  Each core runs a dense [C,H] @ [H,H] matmul in
    float32r (full PE rate, fp32 storage).
  - Host scatters per-expert outputs back to token order and adds p*b.

The per-expert matmul is the only heavy compute: C*H*H*2 ~ 10 GFLOP/core.
"""

import os

import numpy as np

import concourse.bass as bass
import concourse.tile as tile
from concourse import mybir
from concourse.bass_utils import run_bass_kernel_spmd

B, S, H, E = 4, 2048, 2048, 8
P = 128
N_CORES = 8
N_FREE = 512  # matmul moving free dim / PSUM bank width (fp32)

_COMPILED = {}  # capacity -> bass.Bass


def _ensure_ntff_hook() -> bool:
    """Register antenv.axon_hooks with a ctypes NTFF hook if the image lacks it.

    Mirrors trn_agent_boot.trn_boot._ntff_profile_via_ctypes; needed so
    run_bass_kernel_spmd(trace=True) can pull HW profiles under axon.
    """
    import contextlib
    import ctypes
    import sys
    import types

    try:
        from antenv.axon_hooks import get_axon_ntff_profile_hook  # noqa: F401

        return True
    except ImportError:
        pass

    so_path = "/opt/axon/libaxon_pjrt.so"
    if not os.path.exists(so_path):
        return False
    lib = ctypes.CDLL(so_path)
    if not hasattr(lib, "axon_start_nrt_profile"):
        return False
    lib.axon_start_nrt_profile.argtypes = [
        ctypes.POINTER(ctypes.c_int64),
        ctypes.c_size_t,
    ]
    lib.axon_start_nrt_profile.restype = ctypes.c_int64
    lib.axon_stop_nrt_profile.argtypes = [ctypes.c_char_p]
    lib.axon_stop_nrt_profile.restype = ctypes.c_int64

    @contextlib.contextmanager
    def _hook(output_dir, device_ids):
        import jax

        jax.devices()  # force PJRT init so the .so's client exists
        if device_ids:
            ids = (ctypes.c_int64 * len(device_ids))(*device_ids)
            rc = lib.axon_start_nrt_profile(ids, len(device_ids))
        else:
            rc = lib.axon_start_nrt_profile(None, 0)
        if rc != 0:
            raise RuntimeError(f"axon_start_nrt_profile rc={rc}")
        try:
            yield
        finally:
            n = lib.axon_stop_nrt_profile(str(output_dir).encode())
            print(f"ntff profile: {n} file(s) -> {output_dir}")

    import antenv

    mod = types.ModuleType("antenv.axon_hooks")
    mod.get_axon_ntff_profile_hook = lambda: _hook
    mod.set_axon_ntff_profile_hook = lambda h: None
    sys.modules["antenv.axon_hooks"] = mod
    antenv.axon_hooks = mod
    return True


def _build_bass(C: int) -> bass.Bass:
    """SPMD kernel for one core: y[C,H] = xt.T @ w  (xt: [H,C], w: [H,H]).

    Raw bass (no TileContext): explicit semaphores with standalone wait
    instructions sidestep walrus's one-embedded-wait-per-instruction limit
    that Tile's wait assignment keeps overflowing for this dataflow.
    """
    KT = H // P        # 16 k tiles
    MT = (C + P - 1) // P  # token tiles (last may be partial)
    M_LAST = C - (MT - 1) * P
    NT = H // N_FREE   # 4 n tiles
    NPAIR = MT // 2    # paired y stores; plus one single if MT is odd
    f32 = mybir.dt.float32
    f32r = mybir.dt.float32r

    def mcols(mt):
        return P if mt < MT - 1 else M_LAST

    nc = bass.Bass()
    xt = nc.dram_tensor("xt", [H, C], f32r, kind="ExternalInput")
    w = nc.dram_tensor("w", [H, H], f32r, kind="ExternalInput")
    y = nc.dram_tensor("y", [C, H], f32, kind="ExternalOutput")

    xt_r = xt.rearrange("(kt p) c -> p kt c", p=P)  # [128, KT, C]

    with (
        # W fully resident: 128KB/partition.
        nc.sbuf_tensor("w_sb", [P, KT, H], f32r) as w_sb,
        # x tiles 1..MT-1 resident; x tile 0 lives in the pair tile's first
        # half (dead once mt=0's matmuls finish, exactly when the first
        # pair copy overwrites it).  f32r so matmuls may read it; the DVE
        # copies round f32->f32r on the way in (~1e-4 relative).
        nc.sbuf_tensor("x_sb", [P, KT, C - P], f32r) as x_sb,
        nc.sbuf_tensor("pair", [P, 2, H], f32r) as pair,
        nc.psum_tensor("ps0", [P, H], f32) as ps0,
        nc.psum_tensor("ps1", [P, H], f32) as ps1,
        nc.semaphore("sPE") as sPE,
        nc.semaphore("sCopy") as sCopy,
        nc.Block() as block,
    ):
        psums = [ps0, ps1]
        x0_view = pair[:, 0, :].rearrange("p (kt c) -> p kt c", kt=KT)
        sW = [nc.semaphore(f"sW{kt}").__enter__() for kt in range(KT)]
        sX = [nc.semaphore(f"sX{mt}").__enter__() for mt in range(MT)]
        sY = [nc.semaphore(f"sY{j}").__enter__() for j in range(MT)]

        def x_slice(mt, kt):
            if mt == 0:
                return x0_view[:, kt, :]
            return x_sb[:, kt, (mt - 1) * P : (mt - 1) * P + mcols(mt)]

        @block.sync
        def _(sync):
            # Two HWDGE channels (sync + scalar) split the input load so the
            # W stream runs at HBM rate instead of one queue's ~254 GB/s.
            # sync: x0, x1, even W k-tiles, then the rest of x.
            def x_load(eng, mt, nslices=2):
                for s in range(nslices):
                    k0 = s * (KT // nslices)
                    k1 = (s + 1) * (KT // nslices)
                    if mt == 0:
                        eng.dma_start(
                            x0_view[:, k0:k1, :], xt_r[:, k0:k1, 0:P]
                        ).then_inc(sX[0], 16)
                    else:
                        eng.dma_start(
                            x_sb[:, k0:k1, (mt - 1) * P : (mt - 1) * P + mcols(mt)],
                            xt_r[:, k0:k1, mt * P : mt * P + mcols(mt)],
                        ).then_inc(sX[mt], 16)

            x_load(sync, 0, 4)
            x_load(sync, 1, 4)
            for kt in range(0, KT, 2):
                for s in range(NT):  # 0.25MB slices: one DMA engine each
                    sync.dma_start(
                        w_sb[:, kt, s * N_FREE : (s + 1) * N_FREE],
                        w[kt * P : (kt + 1) * P, s * N_FREE : (s + 1) * N_FREE],
                    ).then_inc(sW[kt], 16)
            for mt in range(2, MT):
                x_load(sync, mt)

        @block.scalar
        def _(scalar):
            # scalar: odd W k-tiles, then the y stores (HWDGE, fast) —
            # each store waits its copy, the last m-tile per col-slice.
            for kt in range(1, KT, 2):
                for s in range(NT):
                    scalar.dma_start(
                        w_sb[:, kt, s * N_FREE : (s + 1) * N_FREE],
                        w[kt * P : (kt + 1) * P, s * N_FREE : (s + 1) * N_FREE],
                    ).then_inc(sW[kt], 16)
            for mt in range(MT):
                mc = mcols(mt)
                src_ap = pair[:mc, mt % 2, :].bitcast(f32)
                if mt < MT - 1:
                    scalar.wait_ge(sCopy, mt + 1)
                    for s in range(NT):  # 4 engines per store burst
                        scalar.dma_start(
                            y[mt * P : mt * P + mc, s * N_FREE : (s + 1) * N_FREE],
                            src_ap[:, s * N_FREE : (s + 1) * N_FREE],
                        ).then_inc(sY[mt], 16)
                else:
                    ns = H // 256
                    for s in range(ns):  # last m-tile: 8 narrow slices
                        scalar.wait_ge(sCopy, mt + s // 2 + 1)
                        scalar.dma_start(
                            y[mt * P : mt * P + mc, s * 256 : (s + 1) * 256],
                            src_ap[:, s * 256 : (s + 1) * 256],
                        ).then_inc(sY[mt], 16)
            for mt in range(MT):
                scalar.wait_ge(sY[mt], 64 if mt < MT - 1 else 16 * (H // 256))

        @block.tensor
        def _(tensor):
            def mt_matmuls(mt, kt, psum):
                mc = mcols(mt)
                lhsT = x_slice(mt, kt)
                for nt in range(NT):
                    mm = tensor.matmul(
                        psum[:mc, nt * N_FREE : (nt + 1) * N_FREE],
                        lhsT,
                        w_sb[:, kt, nt * N_FREE : (nt + 1) * N_FREE],
                        start=(kt == 0),
                        stop=(kt == KT - 1),
                        skip_group_check=True,
                    )
                return mm

            # Phase 1: m-tiles 0,1 k-major, chasing the two W DMA streams.
            tensor.wait_ge(sX[0], 64)
            tensor.wait_ge(sX[1], 64)
            for kt in range(KT):
                tensor.wait_ge(sW[kt], 16 * NT)
                for mt in (0, 1):
                    mm = mt_matmuls(mt, kt, psums[mt])
                    if kt == KT - 1:
                        mm.then_inc(sPE, 1)
            # Phase 2: W is resident; stream the remaining m-tiles.
            for mt in range(2, MT):
                tensor.wait_ge(sX[mt], 32)
                tensor.wait_ge(sCopy, mt - 1)  # psum slot free
                for kt in range(KT):
                    mm = mt_matmuls(mt, kt, psums[mt % 2])
                mm.then_inc(sPE, 1)

        @block.vector
        def _(vector):
            for mt in range(MT):
                mc = mcols(mt)
                vector.wait_ge(sPE, mt + 1)
                if mt >= 2:
                    # this pair half is re-written every 2 m-tiles; its
                    # previous y store must have drained
                    vector.wait_ge(sY[mt - 2], 64)
                if mt < MT - 1:
                    vector.tensor_copy(
                        pair[:mc, mt % 2, :], psums[mt % 2][:mc]
                    ).then_inc(sCopy, 1)
                else:
                    # last m-tile: copy in col-slices so the stores pipeline
                    # behind the copies and the kernel tail shrinks
                    for s in range(NT):
                        vector.tensor_copy(
                            pair[:mc, mt % 2, s * N_FREE : (s + 1) * N_FREE],
                            psums[mt % 2][:mc, s * N_FREE : (s + 1) * N_FREE],
                        ).then_inc(sCopy, 1)

    return nc


def _route(x, Wg):
    """Host gating: returns token indices per expert and top-1 probs."""
    xf = np.ascontiguousarray(x.reshape(-1, H))
    logits = xf @ Wg                       # [T, E] fp32 (min top1-top2 gap ~1e-4)
    idx = logits.argmax(-1)
    m = logits.max(-1, keepdims=True)
    ex = np.exp(logits - m)
    p = (ex[np.arange(len(idx)), idx] / ex.sum(-1)).astype(np.float32)
    return xf, idx, p


def _run(inputs, trace=False):
    x = np.asarray(inputs["x"], dtype=np.float32)
    Wg = np.asarray(inputs["Wg"], dtype=np.float32)
    W = np.asarray(inputs["W"], dtype=np.float32)
    b = np.asarray(inputs["b"], dtype=np.float32)

    if trace:
        trace = _ensure_ntff_hook()

    xf, idx, p = _route(x, Wg)
    T = xf.shape[0]

    toks = [np.nonzero(idx == e)[0] for e in range(E)]
    counts = np.array([len(t) for t in toks])
    C = max(P, int(-(-counts.max() // 32) * 32))  # capacity, padded to 32

    if C not in _COMPILED:
        _COMPILED[C] = _build_bass(C)
    nc = _COMPILED[C]

    in_maps = []
    for e in range(E):
        te = toks[e]
        xs = xf[te] * p[te, None]          # fold gate prob into activations
        xt = np.zeros((H, C), dtype=np.float32)
        xt[:, : len(te)] = xs.T
        in_maps.append({"xt": xt, "w": np.ascontiguousarray(W[e])})

    res = run_bass_kernel_spmd(
        nc,
        in_maps,
        core_ids=list(range(N_CORES)),
        trace=trace,
        trace_cores=list(range(N_CORES)) if trace else None,
    )

    out = np.empty((T, H), dtype=np.float32)
    for e in range(E):
        te = toks[e]
        ye = res.results[e]["y"][: len(te)]
        if np.any(b[e]):
            ye = ye + p[te, None] * b[e]
        out[te] = ye
    return out.reshape(B, S, H), res


def kernel(**inputs) -> np.ndarray:
    out, _ = _run(inputs, trace=os.environ.get("MOE_TRACE", "0") == "1")
    return out


def run_traced(inputs):
    """For test.py: returns (output, BassKernelResults with exec_time_ns)."""
    return _run(inputs, trace=True)



# revision 2
# speedup vs baseline: 1.2462x; 1.2462x over previous
"""Top-1 MoE block (B=4, S=2048, H=2048, E=8) for 8 Trainium2 NeuronCores.

Strategy (expert-parallel, host-mediated dispatch):
  - Host computes the tiny gating network (x @ Wg -> softmax -> argmax),
    0.4% of total FLOPs, and the token permutation per expert.
  - Token block for expert e (prob-scaled, cast to bf16, tiled to
    [mt][128 part, 16*128]) plus W[e] (bf16) goes to core e.  Each core
    runs a dense [C,H] @ [H,H] matmul in bf16 (full PE rate, half the
    HBM traffic of fp32).
  - Host upcasts per-expert outputs, scatters back to token order.

Schedule per core:
  - Two HWDGE queues (sync, scalar) stream x tiles 0,1 then W k-tiles
    even/odd as single 0.5MB contiguous DMAs.
  - Tensor engine pre-warms the PE (HAM un-throttle) with junk matmuls
    while the first DMAs land, then chases the W stream k-major over
    m-tiles {0,1} (PSUM holds 2 full-H m-tiles), then streams the
    remaining m-tiles with W resident.
  - Vector engine casts PSUM fp32 -> bf16 y tiles; scalar stores them.
"""

import os

import numpy as np
import ml_dtypes

import concourse.bass as bass
from concourse import mybir
from concourse.bass_utils import run_bass_kernel_spmd

BF16 = ml_dtypes.bfloat16

B, S, H, E = 4, 2048, 2048, 8
P = 128
KT = H // P  # 16 k tiles
N_FREE = 512  # matmul moving free dim / PSUM bank width (fp32)
NT = H // N_FREE  # 4 n tiles
N_CORES = 8
N_WARM = 10  # pre-warm matmuls (N=512 cold ~426ns each)

_COMPILED = {}  # MT -> bass.Bass


def _ensure_ntff_hook() -> bool:
    """Register antenv.axon_hooks with a ctypes NTFF hook if the image lacks it."""
    import contextlib
    import ctypes
    import sys
    import types

    try:
        from antenv.axon_hooks import get_axon_ntff_profile_hook  # noqa: F401

        return True
    except ImportError:
        pass

    so_path = "/opt/axon/libaxon_pjrt.so"
    if not os.path.exists(so_path):
        return False
    lib = ctypes.CDLL(so_path)
    if not hasattr(lib, "axon_start_nrt_profile"):
        return False
    lib.axon_start_nrt_profile.argtypes = [
        ctypes.POINTER(ctypes.c_int64),
        ctypes.c_size_t,
    ]
    lib.axon_start_nrt_profile.restype = ctypes.c_int64
    lib.axon_stop_nrt_profile.argtypes = [ctypes.c_char_p]
    lib.axon_stop_nrt_profile.restype = ctypes.c_int64

    @contextlib.contextmanager
    def _hook(output_dir, device_ids):
        import jax

        jax.devices()  # force PJRT init so the .so's client exists
        if device_ids:
            ids = (ctypes.c_int64 * len(device_ids))(*device_ids)
            rc = lib.axon_start_nrt_profile(ids, len(device_ids))
        else:
            rc = lib.axon_start_nrt_profile(None, 0)
        if rc != 0:
            raise RuntimeError(f"axon_start_nrt_profile rc={rc}")
        try:
            yield
        finally:
            n = lib.axon_stop_nrt_profile(str(output_dir).encode())
            print(f"ntff profile: {n} file(s) -> {output_dir}")

    import antenv

    mod = types.ModuleType("antenv.axon_hooks")
    mod.get_axon_ntff_profile_hook = lambda: _hook
    mod.set_axon_ntff_profile_hook = lambda h: None
    sys.modules["antenv.axon_hooks"] = mod
    antenv.axon_hooks = mod
    return True


def _build_bass(MT: int) -> bass.Bass:
    """SPMD kernel for one core: y[MT*128, H] = x.T @ w in bf16.

    xt: [MT*128, KT*128] where row mt*128+p, col kt*128+t holds
        x_token[mt*128+t, kt*128+p] (i.e. per-m-tile transposed blocks).
    w:  [H, H] row-major.
    y:  [MT*128, H] bf16.
    """
    f32 = mybir.dt.float32
    bf16 = mybir.dt.bfloat16

    nc = bass.Bass()
    xt = nc.dram_tensor("xt", [MT * P, KT * P], bf16, kind="ExternalInput")
    w = nc.dram_tensor("w", [H, H], bf16, kind="ExternalInput")
    y = nc.dram_tensor("y", [MT * P, H], bf16, kind="ExternalOutput")

    with (
        nc.sbuf_tensor("w_sb", [P, KT, H], bf16) as w_sb,
        nc.sbuf_tensor("x_sb", [P, MT, H], bf16) as x_sb,
        nc.sbuf_tensor("y_sb", [P, MT, H], bf16) as y_sb,
        nc.sbuf_tensor("warm", [P, N_FREE], bf16) as warm,
        nc.psum_tensor("ps0", [P, H], f32) as ps0,
        nc.psum_tensor("ps1", [P, H], f32) as ps1,
        nc.semaphore("sPE") as sPE,
        nc.semaphore("sCopy") as sCopy,
        nc.Block() as block,
    ):
        psums = [ps0, ps1]
        sW = [nc.semaphore(f"sW{kt}").__enter__() for kt in range(KT)]
        sX = [nc.semaphore(f"sX{mt}").__enter__() for mt in range(MT)]
        sY = [nc.semaphore(f"sY{mt}").__enter__() for mt in range(MT)]

        @block.sync
        def _(sync):
            # x0, even W k-tiles, then x tiles 2..MT-1.  Each DMA is one
            # contiguous [128 part, 4KB] transfer (full queue rate).
            sync.dma_start(x_sb[:, 0, :], xt[0:P, :]).then_inc(sX[0], 16)
            for kt in range(0, KT, 2):
                sync.dma_start(
                    w_sb[:, kt, :], w[kt * P : (kt + 1) * P, :]
                ).then_inc(sW[kt], 16)
            for mt in range(2, MT):
                sync.dma_start(
                    x_sb[:, mt, :], xt[mt * P : (mt + 1) * P, :]
                ).then_inc(sX[mt], 16)

        @block.scalar
        def _(scalar):
            # x1, odd W k-tiles, then the y stores.
            scalar.dma_start(x_sb[:, 1, :], xt[P : 2 * P, :]).then_inc(sX[1], 16)
            for kt in range(1, KT, 2):
                scalar.dma_start(
                    w_sb[:, kt, :], w[kt * P : (kt + 1) * P, :]
                ).then_inc(sW[kt], 16)
            for mt in range(MT - 1):
                scalar.wait_ge(sCopy, mt + 1)
                scalar.dma_start(
                    y[mt * P : (mt + 1) * P, :], y_sb[:, mt, :]
                ).then_inc(sY[mt], 16)
            # last m-tile: 4 column slices so stores pipeline behind copies
            mt = MT - 1
            for s in range(NT):
                scalar.wait_ge(sCopy, MT + s)
                scalar.dma_start(
                    y[mt * P : (mt + 1) * P, s * N_FREE : (s + 1) * N_FREE],
                    y_sb[:, mt, s * N_FREE : (s + 1) * N_FREE],
                ).then_inc(sY[mt], 16)
            for mt in range(MT - 1):
                scalar.wait_ge(sY[mt], 16)
            scalar.wait_ge(sY[MT - 1], 16 * NT)

        @block.tensor
        def _(tensor):
            def mt_matmuls(mt, kt, psum):
                lhsT = x_sb[:, mt, kt * P : (kt + 1) * P]
                for nt in range(NT):
                    mm = tensor.matmul(
                        psum[:, nt * N_FREE : (nt + 1) * N_FREE],
                        lhsT,
                        w_sb[:, kt, nt * N_FREE : (nt + 1) * N_FREE],
                        start=(kt == 0),
                        stop=(kt == KT - 1),
                        skip_group_check=True,
                    )
                return mm

            # Pre-warm the PE (HAM un-throttles after ~3.4us of activity)
            # on scratch data while the first DMAs land.
            for _ in range(N_WARM):
                tensor.matmul(
                    ps0[:, 0:N_FREE],
                    warm[:, 0:P],
                    warm[:, :],
                    start=True,
                    stop=True,
                    skip_group_check=True,
                )

            # Phase 1: m-tiles 0,1 k-major, chasing the two W DMA streams.
            tensor.wait_ge(sX[0], 16)
            tensor.wait_ge(sX[1], 16)
            for kt in range(KT):
                tensor.wait_ge(sW[kt], 16)
                for mt in (0, 1):
                    mm = mt_matmuls(mt, kt, psums[mt])
                    if kt == KT - 1:
                        mm.then_inc(sPE, 1)
            # Phase 2: W resident; stream the remaining m-tiles.
            for mt in range(2, MT):
                tensor.wait_ge(sX[mt], 16)
                tensor.wait_ge(sCopy, mt - 1)  # psum slot free
                for kt in range(KT):
                    mm = mt_matmuls(mt, kt, psums[mt % 2])
                mm.then_inc(sPE, 1)

        @block.vector
        def _(vector):
            for mt in range(MT):
                vector.wait_ge(sPE, mt + 1)
                if mt < MT - 1:
                    vector.tensor_copy(
                        y_sb[:, mt, :], psums[mt % 2][:, :]
                    ).then_inc(sCopy, 1)
                else:
                    # last m-tile in col-slices so stores pipeline behind
                    # the copies and the kernel tail shrinks
                    for s in range(NT):
                        vector.tensor_copy(
                            y_sb[:, mt, s * N_FREE : (s + 1) * N_FREE],
                            psums[mt % 2][:, s * N_FREE : (s + 1) * N_FREE],
                        ).then_inc(sCopy, 1)

    return nc


def _route(x, Wg):
    """Host gating: returns token indices per expert and top-1 probs."""
    xf = np.ascontiguousarray(x.reshape(-1, H))
    logits = xf @ Wg  # [T, E] fp32 (min top1-top2 gap ~1e-4)
    idx = logits.argmax(-1)
    m = logits.max(-1, keepdims=True)
    ex = np.exp(logits - m)
    p = (ex[np.arange(len(idx)), idx] / ex.sum(-1)).astype(np.float32)
    return xf, idx, p


def _pack_xt(xs: np.ndarray, MT: int) -> np.ndarray:
    """[n_tok, H] fp32 -> [MT*128, KT*128] bf16 per-m-tile transposed tiles."""
    n = xs.shape[0]
    xt = np.zeros((MT * P, KT * P), dtype=BF16)
    for mt in range(MT):
        t0, t1 = mt * P, min((mt + 1) * P, n)
        if t0 >= t1:
            break
        blk = xs[t0:t1].astype(BF16)  # [tc, H]
        # dst row mt*128+p, col kt*128+t  <-  blk[t, kt*128+p]
        tc = t1 - t0
        xt[mt * P : mt * P + P, : tc] = 0  # noop, keeps layout obvious
        dst = xt[mt * P : (mt + 1) * P].reshape(P, KT, P)  # [p, kt, t]
        dst[:, :, :tc] = blk.reshape(tc, KT, P).transpose(2, 1, 0)
    return xt


def _run(inputs, trace=False):
    x = np.asarray(inputs["x"], dtype=np.float32)
    Wg = np.asarray(inputs["Wg"], dtype=np.float32)
    W = np.asarray(inputs["W"], dtype=np.float32)
    b = np.asarray(inputs["b"], dtype=np.float32)

    if trace:
        trace = _ensure_ntff_hook()

    xf, idx, p = _route(x, Wg)
    T = xf.shape[0]

    toks = [np.nonzero(idx == e)[0] for e in range(E)]
    counts = np.array([len(t) for t in toks])
    MT = max(1, int(-(-counts.max() // P)))

    if MT not in _COMPILED:
        _COMPILED[MT] = _build_bass(MT)
    nc = _COMPILED[MT]

    in_maps = []
    for e in range(E):
        te = toks[e]
        xs = xf[te] * p[te, None]  # fold gate prob into activations
        in_maps.append(
            {"xt": _pack_xt(xs, MT), "w": W[e].astype(BF16)}
        )

    res = run_bass_kernel_spmd(
        nc,
        in_maps,
        core_ids=list(range(N_CORES)),
        trace=trace,
        trace_cores=list(range(N_CORES)) if trace else None,
    )

    out = np.empty((T, H), dtype=np.float32)
    for e in range(E):
        te = toks[e]
        ye = res.results[e]["y"][: len(te)].astype(np.float32)
        if np.any(b[e]):
            ye = ye + p[te, None] * b[e]
        out[te] = ye
    return out.reshape(B, S, H), res


def kernel(**inputs) -> np.ndarray:
    out, _ = _run(inputs, trace=os.environ.get("MOE_TRACE", "0") == "1")
    return out


def run_traced(inputs):
    """For test.py: returns (output, BassKernelResults with exec_time_ns)."""
    return _run(inputs, trace=True)


# revision 5
# speedup vs baseline: 1.2616x; 1.0124x over previous
"""Top-1 MoE block (B=4, S=2048, H=2048, E=8) for 8 Trainium2 NeuronCores.

Strategy (expert-parallel, host-mediated dispatch):
  - Host computes the tiny gating network (x @ Wg -> softmax -> argmax),
    0.4% of total FLOPs, and the token permutation per expert.
  - Token block for expert e (prob-scaled, cast to bf16, tiled to
    [mt][128 part, 16*128]) plus W[e] (bf16) goes to core e.  Each core
    runs a dense [C,H] @ [H,H] matmul in bf16 (full PE rate, half the
    HBM traffic of fp32).
  - Host upcasts per-expert outputs, scatters back to token order.

Schedule per core:
  - Two HWDGE queues (sync, scalar) stream x tiles 0,1 then W k-tiles
    even/odd as single 0.5MB contiguous DMAs.
  - Tensor engine pre-warms the PE (HAM un-throttle) with junk matmuls
    while the first DMAs land, then chases the W stream k-major over
    m-tiles {0,1} (PSUM holds 2 full-H m-tiles), then streams the
    remaining m-tiles with W resident.  The last m-tile runs nt-major
    so its cast+store pipeline behind the final matmuls.
  - Vector engine casts PSUM fp32 -> bf16 y tiles; sync+scalar store.
"""

import os

import numpy as np
import ml_dtypes

import concourse.bass as bass
from concourse import mybir
from concourse.bass_utils import run_bass_kernel_spmd

BF16 = ml_dtypes.bfloat16

B, S, H, E = 4, 2048, 2048, 8
P = 128
KT = H // P  # 16 k tiles
N_FREE = 512  # matmul moving free dim / PSUM bank width (fp32)
NT = H // N_FREE  # 4 n tiles
N_CORES = 8
N_WARM = 14  # pre-warm matmuls, N=256 (~213ns cold each)
WARM_N = 256

_COMPILED = {}  # MT -> bass.Bass


def _ensure_ntff_hook() -> bool:
    """Register antenv.axon_hooks with a ctypes NTFF hook if the image lacks it."""
    import contextlib
    import ctypes
    import sys
    import types

    try:
        from antenv.axon_hooks import get_axon_ntff_profile_hook  # noqa: F401

        return True
    except ImportError:
        pass

    so_path = "/opt/axon/libaxon_pjrt.so"
    if not os.path.exists(so_path):
        return False
    lib = ctypes.CDLL(so_path)
    if not hasattr(lib, "axon_start_nrt_profile"):
        return False
    lib.axon_start_nrt_profile.argtypes = [
        ctypes.POINTER(ctypes.c_int64),
        ctypes.c_size_t,
    ]
    lib.axon_start_nrt_profile.restype = ctypes.c_int64
    lib.axon_stop_nrt_profile.argtypes = [ctypes.c_char_p]
    lib.axon_stop_nrt_profile.restype = ctypes.c_int64

    @contextlib.contextmanager
    def _hook(output_dir, device_ids):
        import jax

        jax.devices()  # force PJRT init so the .so's client exists
        if device_ids:
            ids = (ctypes.c_int64 * len(device_ids))(*device_ids)
            rc = lib.axon_start_nrt_profile(ids, len(device_ids))
        else:
            rc = lib.axon_start_nrt_profile(None, 0)
        if rc != 0:
            raise RuntimeError(f"axon_start_nrt_profile rc={rc}")
        try:
            yield
        finally:
            n = lib.axon_stop_nrt_profile(str(output_dir).encode())
            print(f"ntff profile: {n} file(s) -> {output_dir}")

    import antenv

    mod = types.ModuleType("antenv.axon_hooks")
    mod.get_axon_ntff_profile_hook = lambda: _hook
    mod.set_axon_ntff_profile_hook = lambda h: None
    sys.modules["antenv.axon_hooks"] = mod
    antenv.axon_hooks = mod
    return True


def _build_bass(MT: int) -> bass.Bass:
    """SPMD kernel for one core: y[MT*128, H] = x.T @ w in bf16.

    xt: [MT*128, KT*128] where row mt*128+p, col kt*128+t holds
        x_token[mt*128+t, kt*128+p] (i.e. per-m-tile transposed blocks).
    w:  [H, H] row-major.
    y:  [MT*128, H] bf16.
    """
    assert MT >= 3
    f32 = mybir.dt.float32
    bf16 = mybir.dt.bfloat16
    LAST = MT - 1

    nc = bass.Bass()
    xt = nc.dram_tensor("xt", [MT * P, KT * P], bf16, kind="ExternalInput")
    w = nc.dram_tensor("w", [H, H], bf16, kind="ExternalInput")
    y = nc.dram_tensor("y", [MT * P, H], bf16, kind="ExternalOutput")

    # semaphore count plan:
    #   sPE:   mt0 -> +1 per nt (4), mt1..LAST-1 -> +1 each, LAST -> +1 per nt
    #   sCopy: same counting as its copies (mt0 sliced, LAST sliced)
    def spe_done(mt):  # sPE value once m-tile mt fully incremented
        if mt == 0:
            return 4
        return mt + 4 if mt < LAST else LAST + 3 + 4

    with (
        nc.sbuf_tensor("w_sb", [P, KT, H], bf16) as w_sb,
        nc.sbuf_tensor("x_sb", [P, MT, H], bf16) as x_sb,
        nc.sbuf_tensor("y_sb", [P, MT, H], bf16) as y_sb,
        nc.sbuf_tensor("warm", [P, WARM_N], bf16) as warm,
        nc.psum_tensor("ps0", [P, H], f32) as ps0,
        nc.psum_tensor("ps1", [P, H], f32) as ps1,
        nc.semaphore("sPE") as sPE,
        nc.semaphore("sCopy") as sCopy,
        nc.semaphore("sX0a") as sX0a,
        nc.semaphore("sX0b") as sX0b,
        nc.semaphore("sYo") as sYo,
        nc.Block() as block,
    ):
        psums = [ps0, ps1]
        sW = [nc.semaphore(f"sW{kt}").__enter__() for kt in range(KT)]
        sX = [nc.semaphore(f"sX{mt}").__enter__() for mt in range(MT)]
        sY = [nc.semaphore(f"sY{mt}").__enter__() for mt in range(MT)]

        HH = H // 2

        @block.sync
        def _(sync):
            # first half of x0, even W k-tiles, x tiles 2.., even last-tile
            # y stores.
            sync.dma_start(x_sb[:, 0, 0:HH], xt[0:P, 0:HH]).then_inc(sX0a, 16)
            for kt in range(0, KT, 2):
                sync.dma_start(
                    w_sb[:, kt, :], w[kt * P : (kt + 1) * P, :]
                ).then_inc(sW[kt], 16)
            for mt in range(2, MT):
                sync.dma_start(
                    x_sb[:, mt, :], xt[mt * P : (mt + 1) * P, :]
                ).then_inc(sX[mt], 16)
            for s in (0, 2):  # even col-slices of the last m-tile
                sync.wait_ge(sCopy, LAST + 3 + s + 1)
                sync.dma_start(
                    y[LAST * P : (LAST + 1) * P, s * N_FREE : (s + 1) * N_FREE],
                    y_sb[:, LAST, s * N_FREE : (s + 1) * N_FREE],
                ).then_inc(sY[LAST], 16)
            sync.wait_ge(sY[LAST], 32)

        @block.scalar
        def _(scalar):
            # second half of x0, x1, odd W k-tiles, then the y stores.
            scalar.dma_start(x_sb[:, 0, HH:H], xt[0:P, HH:H]).then_inc(sX0b, 16)
            scalar.dma_start(x_sb[:, 1, :], xt[P : 2 * P, :]).then_inc(sX[1], 16)
            for kt in range(1, KT, 2):
                scalar.dma_start(
                    w_sb[:, kt, :], w[kt * P : (kt + 1) * P, :]
                ).then_inc(sW[kt], 16)
            for mt in range(LAST):
                scalar.wait_ge(sCopy, mt + 4)
                scalar.dma_start(
                    y[mt * P : (mt + 1) * P, :], y_sb[:, mt, :]
                ).then_inc(sY[mt], 16)
            for s in (1, 3):  # odd col-slices of the last m-tile
                scalar.wait_ge(sCopy, LAST + 3 + s + 1)
                scalar.dma_start(
                    y[LAST * P : (LAST + 1) * P, s * N_FREE : (s + 1) * N_FREE],
                    y_sb[:, LAST, s * N_FREE : (s + 1) * N_FREE],
                ).then_inc(sYo, 16)
            for mt in range(LAST):
                scalar.wait_ge(sY[mt], 16)
            scalar.wait_ge(sYo, 32)

        @block.tensor
        def _(tensor):
            def mm(psum, mt, kt, nt, start, stop):
                return tensor.matmul(
                    psum[:, nt * N_FREE : (nt + 1) * N_FREE],
                    x_sb[:, mt, kt * P : (kt + 1) * P],
                    w_sb[:, kt, nt * N_FREE : (nt + 1) * N_FREE],
                    start=start,
                    stop=stop,
                    skip_group_check=True,
                )

            # Pre-warm the PE (HAM un-throttles after ~3.4us of activity)
            # on scratch data while the first DMAs land.
            for _ in range(N_WARM):
                tensor.matmul(
                    ps0[:, 0:WARM_N],
                    warm[:, 0:P],
                    warm[:, :],
                    start=True,
                    stop=True,
                    skip_group_check=True,
                )

            # Phase 1: m-tiles 0,1 k-major, chasing the two W DMA streams.
            tensor.wait_ge(sX0a, 16)
            for kt in range(KT):
                if kt == KT // 2:
                    tensor.wait_ge(sX0b, 16)
                tensor.wait_ge(sW[kt], 16)
                for nt in range(NT):
                    m = mm(ps0, 0, kt, nt, kt == 0, kt == KT - 1)
                    if kt == KT - 1:
                        m.then_inc(sPE, 1)  # per-nt: mt0 copy pipelines
                if kt == 0:
                    tensor.wait_ge(sX[1], 16)
                for nt in range(NT):
                    m = mm(ps1, 1, kt, nt, kt == 0, kt == KT - 1)
                if kt == KT - 1:
                    m.then_inc(sPE, 1)
            # Phase 2: W resident; stream the remaining m-tiles.
            for mt in range(2, LAST):
                tensor.wait_ge(sX[mt], 16)
                tensor.wait_ge(sCopy, mt + 2)  # psum slot free
                for kt in range(KT):
                    for nt in range(NT):
                        m = mm(psums[mt % 2], mt, kt, nt, kt == 0, kt == KT - 1)
                m.then_inc(sPE, 1)
            # Last m-tile: nt-major so each col-slice finishes early and
            # its cast+store pipelines behind the remaining matmuls.
            tensor.wait_ge(sX[LAST], 16)
            tensor.wait_ge(sCopy, LAST + 2)
            for nt in range(NT):
                for kt in range(KT):
                    m = mm(psums[LAST % 2], LAST, kt, nt, kt == 0, kt == KT - 1)
                m.then_inc(sPE, 1)

        @block.vector
        def _(vector):
            # m-tile 0: per-nt slices (pipelines with mt1 kt15 matmuls)
            for nt in range(NT):
                vector.wait_ge(sPE, nt + 1)
                vector.tensor_copy(
                    y_sb[:, 0, nt * N_FREE : (nt + 1) * N_FREE],
                    ps0[:, nt * N_FREE : (nt + 1) * N_FREE],
                ).then_inc(sCopy, 1)
            for mt in range(1, LAST):
                vector.wait_ge(sPE, spe_done(mt))
                vector.tensor_copy(
                    y_sb[:, mt, :], psums[mt % 2][:, :]
                ).then_inc(sCopy, 1)
            for nt in range(NT):
                vector.wait_ge(sPE, LAST + 3 + nt + 1)
                vector.tensor_copy(
                    y_sb[:, LAST, nt * N_FREE : (nt + 1) * N_FREE],
                    psums[LAST % 2][:, nt * N_FREE : (nt + 1) * N_FREE],
                ).then_inc(sCopy, 1)

    return nc


def _route(x, Wg):
    """Host gating: returns token indices per expert and top-1 probs."""
    xf = np.ascontiguousarray(x.reshape(-1, H))
    logits = xf @ Wg  # [T, E] fp32 (min top1-top2 gap ~1e-4)
    idx = logits.argmax(-1)
    m = logits.max(-1, keepdims=True)
    ex = np.exp(logits - m)
    p = (ex[np.arange(len(idx)), idx] / ex.sum(-1)).astype(np.float32)
    return xf, idx, p


def _pack_xt(xs: np.ndarray, MT: int) -> np.ndarray:
    """[n_tok, H] fp32 -> [MT*128, KT*128] bf16 per-m-tile transposed tiles."""
    n = xs.shape[0]
    xt = np.zeros((MT * P, KT * P), dtype=BF16)
    for mt in range(MT):
        t0, t1 = mt * P, min((mt + 1) * P, n)
        if t0 >= t1:
            break
        blk = xs[t0:t1].astype(BF16)  # [tc, H]
        tc = t1 - t0
        dst = xt[mt * P : (mt + 1) * P].reshape(P, KT, P)  # [p, kt, t]
        dst[:, :, :tc] = blk.reshape(tc, KT, P).transpose(2, 1, 0)
    return xt


def _run(inputs, trace=False):
    x = np.asarray(inputs["x"], dtype=np.float32)
    Wg = np.asarray(inputs["Wg"], dtype=np.float32)
    W = np.asarray(inputs["W"], dtype=np.float32)
    b = np.asarray(inputs["b"], dtype=np.float32)

    if trace:
        trace = _ensure_ntff_hook()

    xf, idx, p = _route(x, Wg)
    T = xf.shape[0]

    toks = [np.nonzero(idx == e)[0] for e in range(E)]
    counts = np.array([len(t) for t in toks])
    MT = max(3, int(-(-counts.max() // P)))

    if MT not in _COMPILED:
        _COMPILED[MT] = _build_bass(MT)
    nc = _COMPILED[MT]

    in_maps = []
    for e in range(E):
        te = toks[e]
        xs = xf[te] * p[te, None]  # fold gate prob into activations
        in_maps.append({"xt": _pack_xt(xs, MT), "w": W[e].astype(BF16)})

    res = run_bass_kernel_spmd(
        nc,
        in_maps,
        core_ids=list(range(N_CORES)),
        trace=trace,
        trace_cores=list(range(N_CORES)) if trace else None,
    )

    out = np.empty((T, H), dtype=np.float32)
    for e in range(E):
        te = toks[e]
        ye = res.results[e]["y"][: len(te)].astype(np.float32)
        if np.any(b[e]):
            ye = ye + p[te, None] * b[e]
        out[te] = ye
    return out.reshape(B, S, H), res


def kernel(**inputs) -> np.ndarray:
    out, _ = _run(inputs, trace=os.environ.get("MOE_TRACE", "0") == "1")
    return out


def run_traced(inputs):
    """For test.py: returns (output, BassKernelResults with exec_time_ns)."""
    return _run(inputs, trace=True)


# revision 21
# speedup vs baseline: 1.3268x; 1.0517x over previous
"""Top-1 MoE block (B=4, S=2048, H=2048, E=8) for 8 Trainium2 NeuronCores.

Strategy (expert-parallel, host-mediated dispatch):
  - Host computes the tiny gating network (x @ Wg -> softmax -> argmax),
    0.4% of total FLOPs, and the token permutation per expert.
  - Token block for expert e (prob-scaled, cast to bf16, tiled) plus
    W[e] (bf16) goes to core e.  Each core runs a dense matmul in bf16
    (full PE rate, half the HBM traffic of fp32).
  - Tokens beyond 1024 per expert ("overflow", ~210 of 8192 here) are
    packed into one extra half-contraction tile per core: a core pair
    (2g, 2g+1) computes K-halves [0:1024) / [1024:2048) of overflow
    group g; the host sums the two partial outputs.  This keeps every
    core at 8 full m-tiles + 1 half-K tile instead of 9 full tiles.
  - Host upcasts per-expert outputs, scatters back to token order.

Schedule per core:
  - gpsimd (SWDGE) loads x tiles 0,1 while the two HWDGE queues
    (sync, scalar) stream W k-tiles even/odd as 0.5MB contiguous DMAs.
  - Tensor engine pre-warms the PE (HAM un-throttle) with junk matmuls
    while the first DMAs land, then chases the W stream k-major over
    m-tiles {0,1} (m-tile 1 lagging 4 k-tiles so its x and the m-tile-0
    PSUM copy stay off the critical path), then streams the remaining
    m-tiles with W resident.  The final (overflow) tile runs nt-major
    so its casts+stores pipeline behind the last matmuls.
  - Vector (+gpsimd for the final slices) casts PSUM fp32 -> bf16;
    sync+scalar issue the stores.
"""

import os

import numpy as np
import ml_dtypes

import concourse.bass as bass
from concourse import mybir
from concourse.bass_utils import run_bass_kernel_spmd

BF16 = ml_dtypes.bfloat16

B, S, H, E = 4, 2048, 2048, 8
P = 128
KT = H // P  # 16 k tiles
N_FREE = 512  # matmul moving free dim / PSUM bank width (fp32)
NT = H // N_FREE  # 4 n tiles
N_CORES = 8
LAG = 4  # phase-1 m-tile-1 lag (k-tiles)
N_WARM = 25  # pre-warm matmuls, N=256 (~213ns cold each)
WARM_N = 256
CAP = 1024  # main-tile token capacity per core in overflow mode

_COMPILED = {}


def _ensure_ntff_hook() -> bool:
    """Register antenv.axon_hooks with a ctypes NTFF hook if the image lacks it."""
    import contextlib
    import ctypes
    import sys
    import types

    try:
        from antenv.axon_hooks import get_axon_ntff_profile_hook  # noqa: F401

        return True
    except ImportError:
        pass

    so_path = "/opt/axon/libaxon_pjrt.so"
    if not os.path.exists(so_path):
        return False
    lib = ctypes.CDLL(so_path)
    if not hasattr(lib, "axon_start_nrt_profile"):
        return False
    lib.axon_start_nrt_profile.argtypes = [
        ctypes.POINTER(ctypes.c_int64),
        ctypes.c_size_t,
    ]
    lib.axon_start_nrt_profile.restype = ctypes.c_int64
    lib.axon_stop_nrt_profile.argtypes = [ctypes.c_char_p]
    lib.axon_stop_nrt_profile.restype = ctypes.c_int64

    @contextlib.contextmanager
    def _hook(output_dir, device_ids):
        import jax

        jax.devices()  # force PJRT init so the .so's client exists
        if device_ids:
            ids = (ctypes.c_int64 * len(device_ids))(*device_ids)
            rc = lib.axon_start_nrt_profile(ids, len(device_ids))
        else:
            rc = lib.axon_start_nrt_profile(None, 0)
        if rc != 0:
            raise RuntimeError(f"axon_start_nrt_profile rc={rc}")
        try:
            yield
        finally:
            n = lib.axon_stop_nrt_profile(str(output_dir).encode())
            print(f"ntff profile: {n} file(s) -> {output_dir}")

    import antenv

    mod = types.ModuleType("antenv.axon_hooks")
    mod.get_axon_ntff_profile_hook = lambda: _hook
    mod.set_axon_ntff_profile_hook = lambda h: None
    sys.modules["antenv.axon_hooks"] = mod
    antenv.axon_hooks = mod
    return True


def _build_bass(n_main: int, ov: bool) -> bass.Bass:
    """SPMD kernel for one core.

    Main tiles: y[mt] = xt[mt].T @ w for mt in 0..n_main-1 (full K=2048).
    Final tile: ov=True  -> y2 = xt2.T @ w2 with K=1024 (overflow half).
                ov=False -> the last main m-tile (mt = n_main, full K),
                            run nt-major for tail pipelining.

    xt: [MTx*128, KT*128] where row mt*128+p, col kt*128+t holds
        x_token[mt*128+t, kt*128+p] (per-m-tile transposed blocks).
    w:  [H, H] row-major.  y: [MTx*128, H] bf16.
    xt2: [128, 1024], w2: [1024, H], y2: [128, H] (ov mode only).
    """
    assert n_main >= 3
    f32 = mybir.dt.float32
    bf16 = mybir.dt.bfloat16
    MTx = n_main if ov else n_main + 1  # m-tiles in xt/y
    KTF = (H // 2 if ov else H) // P  # k-tiles of the final tile

    nc = bass.Bass()
    xt = nc.dram_tensor("xt", [MTx * P, KT * P], bf16, kind="ExternalInput")
    w = nc.dram_tensor("w", [H, H], bf16, kind="ExternalInput")
    y = nc.dram_tensor("y", [MTx * P, H], bf16, kind="ExternalOutput")
    if ov:
        xt2 = nc.dram_tensor("xt2", [P, KTF * P], bf16, kind="ExternalInput")
        w2 = nc.dram_tensor("w2", [KTF * P, H], bf16, kind="ExternalInput")
        y2 = nc.dram_tensor("y2", [P, H], bf16, kind="ExternalOutput")

    with (
        nc.sbuf_tensor("w_sb", [P, KT, H], bf16) as w_sb,
        nc.sbuf_tensor("x_sb", [P, n_main, H], bf16) as x_sb,
        nc.sbuf_tensor("y_sb", [P, n_main, H], bf16) as y_sb,
        nc.sbuf_tensor("xf_sb", [P, KTF * P], bf16) as xf_sb,
        nc.sbuf_tensor("yf_sb", [P, H], bf16) as yf_sb,
        nc.sbuf_tensor(
            "wf_sb", [P, KTF if ov else 1, H if ov else 2], bf16
        ) as wf_alloc,
        nc.sbuf_tensor("warm", [P, WARM_N], bf16) as warm,
        nc.psum_tensor("ps0", [P, H], f32) as ps0,
        nc.psum_tensor("ps1", [P, H], f32) as ps1,
        nc.semaphore("sPE") as sPE,
        nc.semaphore("sCopy") as sCopy,
        nc.semaphore("sWarm") as sWarm,
        nc.semaphore("sXf") as sXf,
        nc.semaphore("sCLv") as sCLv,
        nc.semaphore("sCLg") as sCLg,
        nc.semaphore("sYsync") as sYsync,
        nc.semaphore("sYscal") as sYscal,
        nc.Block() as block,
    ):
        psums = [ps0, ps1]
        sW = [nc.semaphore(f"sW{kt}").__enter__() for kt in range(KT)]
        sX = [nc.semaphore(f"sX{mt}").__enter__() for mt in range(n_main)]
        sY = [nc.semaphore(f"sY{mt}").__enter__() for mt in range(n_main)]
        ps_f = psums[n_main % 2]
        if ov:
            wf_sb = wf_alloc
            sWf = [nc.semaphore(f"sWf{kt}").__enter__() for kt in range(KTF)]
        else:
            wf_sb, sWf = w_sb, sW  # final tile reuses resident W

        USE_GPSIMD_X = os.environ.get("MOE_GPSIMD_X", "0") == "1"

        if USE_GPSIMD_X:

            @block.gpsimd
            def _(gp):
                # SWDGE: x tiles 0,1 in parallel with the W HWDGE streams.
                gp.dma_start(x_sb[:, 0, :], xt[0:P, :]).then_inc(sX[0], 16)
                gp.dma_start(x_sb[:, 1, :], xt[P : 2 * P, :]).then_inc(sX[1], 16)
                if ov:
                    gp.dma_start(xf_sb[:, :], xt2[:, :]).then_inc(sXf, 16)
                else:
                    gp.dma_start(
                        xf_sb[:, :], xt[n_main * P : (n_main + 1) * P, :]
                    ).then_inc(sXf, 16)


        yf_dst = y2 if ov else y
        r0 = 0 if ov else n_main * P

        @block.sync
        def _(sync):
            # even W k-tiles, x tiles 2.., even w2 tiles, final stores 0,1
            if not USE_GPSIMD_X:
                sync.dma_start(x_sb[:, 0, :], xt[0:P, :]).then_inc(sX[0], 16)
            for kt in range(0, KT, 2):
                sync.dma_start(
                    w_sb[:, kt, :], w[kt * P : (kt + 1) * P, :]
                ).then_inc(sW[kt], 16)
            for mt in range(2, n_main):
                sync.dma_start(
                    x_sb[:, mt, :], xt[mt * P : (mt + 1) * P, :]
                ).then_inc(sX[mt], 16)
            if not USE_GPSIMD_X:
                if ov:
                    sync.dma_start(xf_sb[:, :], xt2[:, :]).then_inc(sXf, 16)
                else:
                    sync.dma_start(
                        xf_sb[:, :], xt[n_main * P : (n_main + 1) * P, :]
                    ).then_inc(sXf, 16)
            if ov:
                for kt in range(0, KTF, 2):
                    sync.dma_start(
                        wf_sb[:, kt, :], w2[kt * P : (kt + 1) * P, :]
                    ).then_inc(sWf[kt], 16)
            for nt in (0, 1):
                sync.wait_ge(sCLv, nt + 1)
                sync.dma_start(
                    yf_dst[r0 : r0 + P, nt * N_FREE : (nt + 1) * N_FREE],
                    yf_sb[:, nt * N_FREE : (nt + 1) * N_FREE],
                ).then_inc(sYsync, 16)
            sync.wait_ge(sYsync, 32)

        @block.scalar
        def _(scalar):
            # odd W k-tiles, odd w2 tiles, main stores, final stores 2,3
            if not USE_GPSIMD_X:
                scalar.dma_start(x_sb[:, 1, :], xt[P : 2 * P, :]).then_inc(
                    sX[1], 16
                )
            for kt in range(1, KT, 2):
                scalar.dma_start(
                    w_sb[:, kt, :], w[kt * P : (kt + 1) * P, :]
                ).then_inc(sW[kt], 16)
            if ov:
                for kt in range(1, KTF, 2):
                    scalar.dma_start(
                        wf_sb[:, kt, :], w2[kt * P : (kt + 1) * P, :]
                    ).then_inc(sWf[kt], 16)
            for mt in range(n_main):
                scalar.wait_ge(sCopy, mt + 1)
                scalar.dma_start(
                    y[mt * P : (mt + 1) * P, :], y_sb[:, mt, :]
                ).then_inc(sY[mt], 16)
            for i, nt in enumerate((2, 3)):
                # ACT casts its own slices then stores them; the sem wait
                # orders the DMA behind the copy's SBUF writes (same-engine
                # issue does NOT imply write completion).
                scalar.wait_ge(sPE, n_main + nt + 1)
                scalar.copy(
                    yf_sb[:, nt * N_FREE : (nt + 1) * N_FREE],
                    ps_f[:, nt * N_FREE : (nt + 1) * N_FREE],
                ).then_inc(sCLg, 1)
                scalar.wait_ge(sCLg, i + 1)
                scalar.dma_start(
                    yf_dst[r0 : r0 + P, nt * N_FREE : (nt + 1) * N_FREE],
                    yf_sb[:, nt * N_FREE : (nt + 1) * N_FREE],
                ).then_inc(sYscal, 16)
            for mt in range(n_main):
                scalar.wait_ge(sY[mt], 16)
            scalar.wait_ge(sYscal, 32)

        @block.tensor
        def _(tensor):
            def mm(psum, mt, kt, nt, start, stop):
                return tensor.matmul(
                    psum[:, nt * N_FREE : (nt + 1) * N_FREE],
                    x_sb[:, mt, kt * P : (kt + 1) * P],
                    w_sb[:, kt, nt * N_FREE : (nt + 1) * N_FREE],
                    start=start,
                    stop=stop,
                    skip_group_check=True,
                )

            def mm4(psum, mt, kt):
                for nt in range(NT):
                    m = mm(psum, mt, kt, nt, kt == 0, kt == KT - 1)
                return m

            # Pre-warm the PE (HAM un-throttles after ~3.4us of activity)
            # on scratch data while the first DMAs land.
            tensor.wait_ge(sWarm, 1)
            for _ in range(N_WARM):
                tensor.matmul(
                    ps0[:, 0:WARM_N],
                    warm[:, 0:P],
                    warm[:, :],
                    start=True,
                    stop=True,
                    skip_group_check=True,
                )

            # Phase 1: m-tiles 0,1 k-major chasing the W DMA streams,
            # m-tile 1 lagging LAG k-tiles.
            tensor.wait_ge(sX[0], 16)
            for kt in range(KT):
                tensor.wait_ge(sW[kt], 16)
                m = mm4(ps0, 0, kt)
                if kt == KT - 1:
                    m.then_inc(sPE, 1)
                if kt == LAG:
                    tensor.wait_ge(sX[1], 16)
                if kt >= LAG:
                    mm4(ps1, 1, kt - LAG)
            for kt in range(KT - LAG, KT):
                m = mm4(ps1, 1, kt)
            m.then_inc(sPE, 1)
            # Phase 2: W resident; stream the remaining m-tiles.
            for mt in range(2, n_main):
                tensor.wait_ge(sX[mt], 16)
                tensor.wait_ge(sCopy, mt - 1)  # psum slot free
                for kt in range(KT):
                    m = mm4(psums[mt % 2], mt, kt)
                m.then_inc(sPE, 1)
            # Final tile: nt-major so each col-slice finishes early and
            # its cast+store pipelines behind the remaining matmuls.
            tensor.wait_ge(sXf, 16)
            tensor.wait_ge(sCopy, n_main - 1)
            for nt in range(NT):
                for kt in range(KTF):
                    if nt == 0:
                        tensor.wait_ge(sWf[kt], 16)
                    m = tensor.matmul(
                        ps_f[:, nt * N_FREE : (nt + 1) * N_FREE],
                        xf_sb[:, kt * P : (kt + 1) * P],
                        wf_sb[:, kt, nt * N_FREE : (nt + 1) * N_FREE],
                        start=(kt == 0),
                        stop=(kt == KTF - 1),
                        skip_group_check=True,
                    )
                m.then_inc(sPE, 1)

        @block.vector
        def _(vector):
            vector.memset(warm[:, :], 0.25).then_inc(sWarm, 1)
            for mt in range(n_main):
                vector.wait_ge(sPE, mt + 1)
                vector.tensor_copy(
                    y_sb[:, mt, :], psums[mt % 2][:, :]
                ).then_inc(sCopy, 1)
            for nt in (0, 1):
                vector.wait_ge(sPE, n_main + nt + 1)
                vector.tensor_copy(
                    yf_sb[:, nt * N_FREE : (nt + 1) * N_FREE],
                    ps_f[:, nt * N_FREE : (nt + 1) * N_FREE],
                ).then_inc(sCLv, 1)

    return nc


def _route(x, Wg):
    """Host gating: returns token indices per expert and top-1 probs."""
    xf = np.ascontiguousarray(x.reshape(-1, H))
    logits = xf @ Wg  # [T, E] fp32 (min top1-top2 gap ~1e-4)
    idx = logits.argmax(-1)
    m = logits.max(-1, keepdims=True)
    ex = np.exp(logits - m)
    p = (ex[np.arange(len(idx)), idx] / ex.sum(-1)).astype(np.float32)
    return xf, idx, p


def _pack_tiles(xs: np.ndarray, n_tiles: int, k: int) -> np.ndarray:
    """[n_tok, k] fp32 -> [n_tiles*128, k] bf16 per-m-tile transposed tiles.

    Row mt*128+p, col kt*128+t  <-  xs[mt*128+t, kt*128+p].
    """
    n = xs.shape[0]
    kt = k // P
    out = np.zeros((n_tiles * P, k), dtype=BF16)
    for mt in range(n_tiles):
        t0, t1 = mt * P, min((mt + 1) * P, n)
        if t0 >= t1:
            break
        blk = xs[t0:t1].astype(BF16)  # [tc, k]
        tc = t1 - t0
        dst = out[mt * P : (mt + 1) * P].reshape(P, kt, P)  # [p, kt, t]
        dst[:, :, :tc] = blk.reshape(tc, kt, P).transpose(2, 1, 0)
    return out


def _run(inputs, trace=False):
    x = np.asarray(inputs["x"], dtype=np.float32)
    Wg = np.asarray(inputs["Wg"], dtype=np.float32)
    W = np.asarray(inputs["W"], dtype=np.float32)
    b = np.asarray(inputs["b"], dtype=np.float32)

    if trace:
        trace = _ensure_ntff_hook()

    xf, idx, p = _route(x, Wg)
    T = xf.shape[0]

    toks = [np.nonzero(idx == e)[0] for e in range(E)]
    counts = np.array([len(t) for t in toks])

    # Overflow pieces: per-expert token chunks beyond CAP, each <= 128.
    pieces = []
    for e in range(E):
        o = toks[e][CAP:]
        for i in range(0, len(o), P):
            pieces.append((e, o[i : i + P]))

    ov = 0 < len(pieces) <= N_CORES // 2 and counts.max() > CAP
    if ov:
        n_main = CAP // P
        key = ("OV", n_main)
    else:
        n_main = max(3, int(-(-counts.max() // P)) - 1)
        key = ("A", n_main)
    if key not in _COMPILED:
        _COMPILED[key] = _build_bass(n_main, ov)
    nc = _COMPILED[key]

    KH = H // 2
    in_maps = []
    for c in range(N_CORES):
        e = c
        te = toks[e][: CAP if ov else None]
        xs = xf[te] * p[te, None]  # fold gate prob into activations
        m = {
            "xt": _pack_tiles(xs, n_main if ov else n_main + 1, H),
            "w": W[e].astype(BF16),
        }
        if ov:
            g, h = c // 2, c % 2
            if g < len(pieces):
                e2, t2 = pieces[g]
                xs2 = (xf[t2] * p[t2, None])[:, h * KH : (h + 1) * KH]
                m["xt2"] = _pack_tiles(xs2, 1, KH)
                m["w2"] = W[e2][h * KH : (h + 1) * KH].astype(BF16)
            else:
                m["xt2"] = np.zeros((P, KH), dtype=BF16)
                m["w2"] = np.zeros((KH, H), dtype=BF16)
        in_maps.append(m)

    res = run_bass_kernel_spmd(
        nc,
        in_maps,
        core_ids=list(range(N_CORES)),
        trace=trace,
        trace_cores=list(range(N_CORES)) if trace else None,
    )

    out = np.empty((T, H), dtype=np.float32)
    for e in range(E):
        te = toks[e][: CAP if ov else None]
        ye = res.results[e]["y"][: len(te)].astype(np.float32)
        if np.any(b[e]):
            ye = ye + p[te, None] * b[e]
        out[te] = ye
    if ov:
        for g in range(len(pieces)):
            e2, t2 = pieces[g]
            ye = (
                res.results[2 * g]["y2"][: len(t2)].astype(np.float32)
                + res.results[2 * g + 1]["y2"][: len(t2)].astype(np.float32)
            )
            if np.any(b[e2]):
                ye = ye + p[t2, None] * b[e2]
            out[t2] = ye
    return out.reshape(B, S, H), res


def kernel(**inputs) -> np.ndarray:
    out, _ = _run(inputs, trace=os.environ.get("MOE_TRACE", "0") == "1")
    return out


def run_traced(inputs):
    """For test.py: returns (output, BassKernelResults with exec_time_ns)."""
    return _run(inputs, trace=True)


# revision 26
# speedup vs baseline: 1.3313x; 1.0034x over previous
"""Top-1 MoE block (B=4, S=2048, H=2048, E=8) for 8 Trainium2 NeuronCores.

Strategy (expert-parallel, host-mediated dispatch):
  - Host computes the tiny gating network (x @ Wg -> softmax -> argmax),
    0.4% of total FLOPs, and the token permutation per expert.
  - Token block for expert e (prob-scaled, cast to bf16, tiled) plus
    W[e] (bf16) goes to core e.  Each core runs a dense matmul in bf16
    (full PE rate, half the HBM traffic of fp32).
  - Tokens beyond 1024 per expert ("overflow", ~210 of 8192 here) are
    packed into one extra half-contraction tile per core: a core pair
    (2g, 2g+1) computes K-halves [0:1024) / [1024:2048) of overflow
    group g; the host sums the two partial outputs.  This keeps every
    core at 8 full m-tiles + 1 half-K tile instead of 9 full tiles.
  - Host upcasts per-expert outputs, scatters back to token order.

Schedule per core:
  - gpsimd (SWDGE) loads x tiles 0,1 while the two HWDGE queues
    (sync, scalar) stream W k-tiles even/odd as 0.5MB contiguous DMAs.
  - Tensor engine pre-warms the PE (HAM un-throttle) with junk matmuls
    while the first DMAs land, then chases the W stream k-major over
    m-tiles {0,1} (m-tile 1 lagging 4 k-tiles so its x and the m-tile-0
    PSUM copy stay off the critical path), then streams the remaining
    m-tiles with W resident.  The final (overflow) tile runs nt-major
    so its casts+stores pipeline behind the last matmuls.
  - Vector (+gpsimd for the final slices) casts PSUM fp32 -> bf16;
    sync+scalar issue the stores.
"""

import os

import numpy as np
import ml_dtypes

import concourse.bass as bass
from concourse import mybir
from concourse.bass_utils import run_bass_kernel_spmd

BF16 = ml_dtypes.bfloat16

B, S, H, E = 4, 2048, 2048, 8
P = 128
KT = H // P  # 16 k tiles
N_FREE = 512  # matmul moving free dim / PSUM bank width (fp32)
NT = H // N_FREE  # 4 n tiles
N_CORES = 8
LAG = 1  # phase-1 m-tile-1 lag (k-tiles)
N_WARM = 40  # pre-warm matmuls, N=256 (~213ns cold / ~110ns warm each)
WARM_N = 256
CAP = 1024  # main-tile token capacity per core in overflow mode

_COMPILED = {}


def _ensure_ntff_hook() -> bool:
    """Register antenv.axon_hooks with a ctypes NTFF hook if the image lacks it."""
    import contextlib
    import ctypes
    import sys
    import types

    try:
        from antenv.axon_hooks import get_axon_ntff_profile_hook  # noqa: F401

        return True
    except ImportError:
        pass

    so_path = "/opt/axon/libaxon_pjrt.so"
    if not os.path.exists(so_path):
        return False
    lib = ctypes.CDLL(so_path)
    if not hasattr(lib, "axon_start_nrt_profile"):
        return False
    lib.axon_start_nrt_profile.argtypes = [
        ctypes.POINTER(ctypes.c_int64),
        ctypes.c_size_t,
    ]
    lib.axon_start_nrt_profile.restype = ctypes.c_int64
    lib.axon_stop_nrt_profile.argtypes = [ctypes.c_char_p]
    lib.axon_stop_nrt_profile.restype = ctypes.c_int64

    @contextlib.contextmanager
    def _hook(output_dir, device_ids):
        import jax

        jax.devices()  # force PJRT init so the .so's client exists
        if device_ids:
            ids = (ctypes.c_int64 * len(device_ids))(*device_ids)
            rc = lib.axon_start_nrt_profile(ids, len(device_ids))
        else:
            rc = lib.axon_start_nrt_profile(None, 0)
        if rc != 0:
            raise RuntimeError(f"axon_start_nrt_profile rc={rc}")
        try:
            yield
        finally:
            n = lib.axon_stop_nrt_profile(str(output_dir).encode())
            print(f"ntff profile: {n} file(s) -> {output_dir}")

    import antenv

    mod = types.ModuleType("antenv.axon_hooks")
    mod.get_axon_ntff_profile_hook = lambda: _hook
    mod.set_axon_ntff_profile_hook = lambda h: None
    sys.modules["antenv.axon_hooks"] = mod
    antenv.axon_hooks = mod
    return True


def _build_bass(n_main: int, ov: bool) -> bass.Bass:
    """SPMD kernel for one core.

    Main tiles: y[mt] = xt[mt].T @ w for mt in 0..n_main-1 (full K=2048).
    Final tile: ov=True  -> y2 = xt2.T @ w2 with K=1024 (overflow half).
                ov=False -> the last main m-tile (mt = n_main, full K),
                            run nt-major for tail pipelining.

    xt: [MTx*128, KT*128] where row mt*128+p, col kt*128+t holds
        x_token[mt*128+t, kt*128+p] (per-m-tile transposed blocks).
    w:  [H, H] row-major.  y: [MTx*128, H] bf16.
    xt2: [128, 1024], w2: [1024, H], y2: [128, H] (ov mode only).
    """
    assert n_main >= 3
    f32 = mybir.dt.float32
    bf16 = mybir.dt.bfloat16
    MTx = n_main if ov else n_main + 1  # m-tiles in xt/y
    KTF = (H // 2 if ov else H) // P  # k-tiles of the final tile

    nc = bass.Bass()
    xt = nc.dram_tensor("xt", [MTx * P, KT * P], bf16, kind="ExternalInput")
    w = nc.dram_tensor("w", [H, H], bf16, kind="ExternalInput")
    y = nc.dram_tensor("y", [MTx * P, H], bf16, kind="ExternalOutput")
    if ov:
        xt2 = nc.dram_tensor("xt2", [P, KTF * P], bf16, kind="ExternalInput")
        w2 = nc.dram_tensor("w2", [KTF * P, H], bf16, kind="ExternalInput")
        y2 = nc.dram_tensor("y2", [P, H], bf16, kind="ExternalOutput")

    with (
        nc.sbuf_tensor("w_sb", [P, KT, H], bf16) as w_sb,
        nc.sbuf_tensor("x_sb", [P, n_main, H], bf16) as x_sb,
        nc.sbuf_tensor("y_sb", [P, n_main, H], bf16) as y_sb,
        nc.sbuf_tensor("xf_sb", [P, KTF * P], bf16) as xf_sb,
        nc.sbuf_tensor("yf_sb", [P, H], bf16) as yf_sb,
        nc.sbuf_tensor(
            "wf_sb", [P, KTF if ov else 1, H if ov else 2], bf16
        ) as wf_alloc,
        nc.sbuf_tensor("warm", [P, WARM_N], bf16) as warm,
        nc.psum_tensor("ps0", [P, H], f32) as ps0,
        nc.psum_tensor("ps1", [P, H], f32) as ps1,
        nc.semaphore("sPE") as sPE,
        nc.semaphore("sCopy") as sCopy,
        nc.semaphore("sWarm") as sWarm,
        nc.semaphore("sXf") as sXf,
        nc.semaphore("sCLv") as sCLv,
        nc.semaphore("sCLg") as sCLg,
        nc.semaphore("sYsync") as sYsync,
        nc.semaphore("sYscal") as sYscal,
        nc.Block() as block,
    ):
        psums = [ps0, ps1]
        sW = [nc.semaphore(f"sW{kt}").__enter__() for kt in range(KT)]
        sX = [nc.semaphore(f"sX{mt}").__enter__() for mt in range(n_main)]
        sY = [nc.semaphore(f"sY{mt}").__enter__() for mt in range(n_main)]
        ps_f = psums[n_main % 2]
        if ov:
            wf_sb = wf_alloc
            sWf = [nc.semaphore(f"sWf{kt}").__enter__() for kt in range(KTF)]
        else:
            wf_sb, sWf = w_sb, sW  # final tile reuses resident W

        USE_GPSIMD_X = os.environ.get("MOE_GPSIMD_X", "0") == "1"

        if USE_GPSIMD_X:

            @block.gpsimd
            def _(gp):
                # SWDGE: x tiles 0,1 in parallel with the W HWDGE streams.
                gp.dma_start(x_sb[:, 0, :], xt[0:P, :]).then_inc(sX[0], 16)
                gp.dma_start(x_sb[:, 1, :], xt[P : 2 * P, :]).then_inc(sX[1], 16)
                if ov:
                    gp.dma_start(xf_sb[:, :], xt2[:, :]).then_inc(sXf, 16)
                else:
                    gp.dma_start(
                        xf_sb[:, :], xt[n_main * P : (n_main + 1) * P, :]
                    ).then_inc(sXf, 16)


        yf_dst = y2 if ov else y
        r0 = 0 if ov else n_main * P

        @block.sync
        def _(sync):
            # even W k-tiles, x tiles 2.., even w2 tiles, final stores 0,1
            if not USE_GPSIMD_X:
                sync.dma_start(x_sb[:, 0, :], xt[0:P, :]).then_inc(sX[0], 16)
            for kt in range(0, KT, 2):
                sync.dma_start(
                    w_sb[:, kt, :], w[kt * P : (kt + 1) * P, :]
                ).then_inc(sW[kt], 16)
            for mt in range(2, n_main):
                sync.dma_start(
                    x_sb[:, mt, :], xt[mt * P : (mt + 1) * P, :]
                ).then_inc(sX[mt], 16)
            if not USE_GPSIMD_X:
                if ov:
                    sync.dma_start(xf_sb[:, :], xt2[:, :]).then_inc(sXf, 16)
                else:
                    sync.dma_start(
                        xf_sb[:, :], xt[n_main * P : (n_main + 1) * P, :]
                    ).then_inc(sXf, 16)
            if ov:
                for kt in range(0, KTF, 2):
                    sync.dma_start(
                        wf_sb[:, kt, :], w2[kt * P : (kt + 1) * P, :]
                    ).then_inc(sWf[kt], 16)
            for i, nt in enumerate((0, 1, 3)):
                sync.wait_ge(sCLv, i + 1)
                sync.dma_start(
                    yf_dst[r0 : r0 + P, nt * N_FREE : (nt + 1) * N_FREE],
                    yf_sb[:, nt * N_FREE : (nt + 1) * N_FREE],
                ).then_inc(sYsync, 16)
            sync.wait_ge(sYsync, 48)

        @block.scalar
        def _(scalar):
            # odd W k-tiles, odd w2 tiles, main stores, final stores 2,3
            if not USE_GPSIMD_X:
                scalar.dma_start(x_sb[:, 1, :], xt[P : 2 * P, :]).then_inc(
                    sX[1], 16
                )
            for kt in range(1, KT, 2):
                scalar.dma_start(
                    w_sb[:, kt, :], w[kt * P : (kt + 1) * P, :]
                ).then_inc(sW[kt], 16)
            if ov:
                for kt in range(1, KTF, 2):
                    scalar.dma_start(
                        wf_sb[:, kt, :], w2[kt * P : (kt + 1) * P, :]
                    ).then_inc(sWf[kt], 16)
            for mt in range(n_main):
                scalar.wait_ge(sCopy, mt + 1)
                scalar.dma_start(
                    y[mt * P : (mt + 1) * P, :], y_sb[:, mt, :]
                ).then_inc(sY[mt], 16)
            for i, nt in enumerate((2,)):
                # ACT casts its own slice then stores it; the sem wait
                # orders the DMA behind the copy's SBUF writes (same-engine
                # issue does NOT imply write completion).
                scalar.wait_ge(sPE, n_main + nt + 1)
                scalar.copy(
                    yf_sb[:, nt * N_FREE : (nt + 1) * N_FREE],
                    ps_f[:, nt * N_FREE : (nt + 1) * N_FREE],
                ).then_inc(sCLg, 1)
                scalar.wait_ge(sCLg, i + 1)
                scalar.dma_start(
                    yf_dst[r0 : r0 + P, nt * N_FREE : (nt + 1) * N_FREE],
                    yf_sb[:, nt * N_FREE : (nt + 1) * N_FREE],
                ).then_inc(sYscal, 16)
            for mt in range(n_main):
                scalar.wait_ge(sY[mt], 16)
            scalar.wait_ge(sYscal, 16)

        @block.tensor
        def _(tensor):
            def mm(psum, mt, kt, nt, start, stop):
                return tensor.matmul(
                    psum[:, nt * N_FREE : (nt + 1) * N_FREE],
                    x_sb[:, mt, kt * P : (kt + 1) * P],
                    w_sb[:, kt, nt * N_FREE : (nt + 1) * N_FREE],
                    start=start,
                    stop=stop,
                    skip_group_check=True,
                )

            def mm4(psum, mt, kt):
                for nt in range(NT):
                    m = mm(psum, mt, kt, nt, kt == 0, kt == KT - 1)
                return m

            # Pre-warm the PE (HAM un-throttles after ~3.4us of activity)
            # on scratch data while the first DMAs land.
            tensor.wait_ge(sWarm, 1)
            for _ in range(N_WARM):
                tensor.matmul(
                    ps0[:, 0:WARM_N],
                    warm[:, 0:P],
                    warm[:, :],
                    start=True,
                    stop=True,
                    skip_group_check=True,
                )

            # Phase 1: m-tiles 0,1 k-major chasing the W DMA streams,
            # m-tile 1 lagging LAG k-tiles.
            tensor.wait_ge(sX[0], 16)
            for kt in range(KT):
                tensor.wait_ge(sW[kt], 16)
                m = mm4(ps0, 0, kt)
                if kt == KT - 1:
                    m.then_inc(sPE, 1)
                if kt == LAG:
                    tensor.wait_ge(sX[1], 16)
                if kt >= LAG:
                    mm4(ps1, 1, kt - LAG)
            for kt in range(KT - LAG, KT):
                m = mm4(ps1, 1, kt)
            m.then_inc(sPE, 1)
            # Phase 2: W resident; stream the remaining m-tiles.
            for mt in range(2, n_main):
                tensor.wait_ge(sX[mt], 16)
                tensor.wait_ge(sCopy, mt - 1)  # psum slot free
                for kt in range(KT):
                    m = mm4(psums[mt % 2], mt, kt)
                m.then_inc(sPE, 1)
            # Final tile: nt-major so each col-slice finishes early and
            # its cast+store pipelines behind the remaining matmuls.
            tensor.wait_ge(sXf, 16)
            tensor.wait_ge(sCopy, n_main - 1)
            for nt in range(NT):
                for kt in range(KTF):
                    if nt == 0:
                        tensor.wait_ge(sWf[kt], 16)
                    m = tensor.matmul(
                        ps_f[:, nt * N_FREE : (nt + 1) * N_FREE],
                        xf_sb[:, kt * P : (kt + 1) * P],
                        wf_sb[:, kt, nt * N_FREE : (nt + 1) * N_FREE],
                        start=(kt == 0),
                        stop=(kt == KTF - 1),
                        skip_group_check=True,
                    )
                m.then_inc(sPE, 1)

        @block.vector
        def _(vector):
            vector.memset(warm[:, :], 0.25).then_inc(sWarm, 1)
            for mt in range(n_main):
                vector.wait_ge(sPE, mt + 1)
                vector.tensor_copy(
                    y_sb[:, mt, :], psums[mt % 2][:, :]
                ).then_inc(sCopy, 1)
            for nt in (0, 1, 3):
                vector.wait_ge(sPE, n_main + nt + 1)
                vector.tensor_copy(
                    yf_sb[:, nt * N_FREE : (nt + 1) * N_FREE],
                    ps_f[:, nt * N_FREE : (nt + 1) * N_FREE],
                ).then_inc(sCLv, 1)

    return nc


def _route(x, Wg):
    """Host gating: returns token indices per expert and top-1 probs."""
    xf = np.ascontiguousarray(x.reshape(-1, H))
    logits = xf @ Wg  # [T, E] fp32 (min top1-top2 gap ~1e-4)
    idx = logits.argmax(-1)
    m = logits.max(-1, keepdims=True)
    ex = np.exp(logits - m)
    p = (ex[np.arange(len(idx)), idx] / ex.sum(-1)).astype(np.float32)
    return xf, idx, p


def _pack_tiles(xs: np.ndarray, n_tiles: int, k: int) -> np.ndarray:
    """[n_tok, k] fp32 -> [n_tiles*128, k] bf16 per-m-tile transposed tiles.

    Row mt*128+p, col kt*128+t  <-  xs[mt*128+t, kt*128+p].
    """
    n = xs.shape[0]
    kt = k // P
    out = np.zeros((n_tiles * P, k), dtype=BF16)
    for mt in range(n_tiles):
        t0, t1 = mt * P, min((mt + 1) * P, n)
        if t0 >= t1:
            break
        blk = xs[t0:t1].astype(BF16)  # [tc, k]
        tc = t1 - t0
        dst = out[mt * P : (mt + 1) * P].reshape(P, kt, P)  # [p, kt, t]
        dst[:, :, :tc] = blk.reshape(tc, kt, P).transpose(2, 1, 0)
    return out


def _run(inputs, trace=False):
    x = np.asarray(inputs["x"], dtype=np.float32)
    Wg = np.asarray(inputs["Wg"], dtype=np.float32)
    W = np.asarray(inputs["W"], dtype=np.float32)
    b = np.asarray(inputs["b"], dtype=np.float32)

    if trace:
        trace = _ensure_ntff_hook()

    xf, idx, p = _route(x, Wg)
    T = xf.shape[0]

    toks = [np.nonzero(idx == e)[0] for e in range(E)]
    counts = np.array([len(t) for t in toks])

    # Overflow pieces: per-expert token chunks beyond CAP, each <= 128.
    pieces = []
    for e in range(E):
        o = toks[e][CAP:]
        for i in range(0, len(o), P):
            pieces.append((e, o[i : i + P]))

    ov = 0 < len(pieces) <= N_CORES // 2 and counts.max() > CAP
    if ov:
        n_main = CAP // P
        key = ("OV", n_main)
    else:
        n_main = max(3, int(-(-counts.max() // P)) - 1)
        key = ("A", n_main)
    if key not in _COMPILED:
        _COMPILED[key] = _build_bass(n_main, ov)
    nc = _COMPILED[key]

    KH = H // 2
    in_maps = []
    for c in range(N_CORES):
        e = c
        te = toks[e][: CAP if ov else None]
        xs = xf[te] * p[te, None]  # fold gate prob into activations
        m = {
            "xt": _pack_tiles(xs, n_main if ov else n_main + 1, H),
            "w": W[e].astype(BF16),
        }
        if ov:
            g, h = c // 2, c % 2
            if g < len(pieces):
                e2, t2 = pieces[g]
                xs2 = (xf[t2] * p[t2, None])[:, h * KH : (h + 1) * KH]
                m["xt2"] = _pack_tiles(xs2, 1, KH)
                m["w2"] = W[e2][h * KH : (h + 1) * KH].astype(BF16)
            else:
                m["xt2"] = np.zeros((P, KH), dtype=BF16)
                m["w2"] = np.zeros((KH, H), dtype=BF16)
        in_maps.append(m)

    res = run_bass_kernel_spmd(
        nc,
        in_maps,
        core_ids=list(range(N_CORES)),
        trace=trace,
        trace_cores=list(range(N_CORES)) if trace else None,
    )

    out = np.empty((T, H), dtype=np.float32)
    for e in range(E):
        te = toks[e][: CAP if ov else None]
        ye = res.results[e]["y"][: len(te)].astype(np.float32)
        if np.any(b[e]):
            ye = ye + p[te, None] * b[e]
        out[te] = ye
    if ov:
        for g in range(len(pieces)):
            e2, t2 = pieces[g]
            ye = (
                res.results[2 * g]["y2"][: len(t2)].astype(np.float32)
                + res.results[2 * g + 1]["y2"][: len(t2)].astype(np.float32)
            )
            if np.any(b[e2]):
                ye = ye + p[t2, None] * b[e2]
            out[t2] = ye
    return out.reshape(B, S, H), res


def kernel(**inputs) -> np.ndarray:
    out, _ = _run(inputs, trace=os.environ.get("MOE_TRACE", "0") == "1")
    return out


def run_traced(inputs):
    """For test.py: returns (output, BassKernelResults with exec_time_ns)."""
    return _run(inputs, trace=True)
